# revision 23
# baseline (speedup 1.0000x reference)
"""Trainium2 Bass kernel for nn_NewModel_42356967473589 (dense_transformer).

Model: two BiAttention blocks + final linear mapping.
  o = BiAttn(ctx, q1) ; o = BiAttn(o, q2) ; out = o @ w_map.T + b_map

Sharding: 8 cores = (batch b in 0..3) x (context half h in 0..1).
Each core owns 1024 context rows of one batch. All compute is row-local
except the softmax-over-context (weight_two); its (sum-exp, weighted-sum)
stats are combined across the pair of cores sharing a batch via a tiny
pairwise AllReduce, overlapped with the large matmuls.

Math restructure (per stage, X = stage input [C,D], M = memory [Q,D]):
  out = X@W1 + o1@W2 + (X*o1)@W3 + (t*o1)@W4      (W_k = w_out[:, kD:(k+1)D].T)
  o1 = P@M (rank Q=64), t broadcast over rows =>
  o1@W2 + (t*o1)@W4 = P @ (M @ (W2 + t*W4))        (rank-64 path)

v3: all heavy matmul operands bf16 (psum fp32); softmax reciprocal in a
[128,4] column layout (DVE reciprocal is 8 cyc/elem/lane, so [1,512] on
one partition costs 4.3us vs ~0.2us here), row-broadcast back via a
stride-0-lhsT identity matmul.  PE queue is kept dense end-to-end (HAM
clock gate re-throttles after idle): o1 matmuls interleave with the
W1-parts of the first output groups, the rank-64 correction of the last
two j-blocks rides inside their psum accumulation groups, and weight
DMAs are dependency-gated on the gpsimd queue so the stage-1 input
transfer gets full HBM bandwidth at startup.
"""

import numpy as np
import ml_dtypes

import concourse.bacc as bacc
import concourse.tile as tile
from concourse import mybir
from concourse.bass_utils import run_bass_kernel_spmd
from contextlib import ExitStack
import bass_rust

f32 = mybir.dt.float32
f32r = mybir.dt.float32r
bf16 = mybir.dt.bfloat16
i32 = mybir.dt.int32
Alu = mybir.AluOpType
AF = bass_rust.ActivationFunctionType
AX = bass_rust.AxisListType
RedOp = bass_rust.ReduceOp

B, C_LEN, Q_LEN, D = 4, 2048, 64, 1024
N_CORES = 8
R = C_LEN // 2          # rows per core
NK = D // 128           # contraction chunks
RH = R // 512           # row halves (moving-dim tiles)
D2 = 2 * D
NEGBIG = 10000.0

_CACHED_NC = None


def _build_nc():
    nc = bacc.Bacc("TRN2", target_bir_lowering=False, debug=False,
                   num_devices=N_CORES)

    # ---- per-core DRAM I/O (host pre-tiled layouts, see _shard_inputs) ----
    xt_ap = nc.dram_tensor("xt", [128, NK * R], bf16, kind="ExternalInput").ap()
    m_t = [nc.dram_tensor(f"m{s}t", [128, NK * Q_LEN], bf16, kind="ExternalInput").ap() for s in (1, 2)]
    m_n = [nc.dram_tensor(f"m{s}n", [Q_LEN, D], bf16, kind="ExternalInput").ap() for s in (1, 2)]
    vecb = [nc.dram_tensor(f"vecb{s}", [128, NK * 2], bf16, kind="ExternalInput").ap() for s in (1, 2)]
    sclf = [nc.dram_tensor(f"sclf{s}", [128, NK], f32, kind="ExternalInput").ap() for s in (1, 2)]
    msk = [nc.dram_tensor(f"mask{s}", [Q_LEN, 1], i32, kind="ExternalInput").ap() for s in (1, 2)]
    w13 = [nc.dram_tensor(f"w13_{s}", [128, NK * 2 * NK * 128], bf16, kind="ExternalInput").ap() for s in (1, 2)]
    w24 = [nc.dram_tensor(f"w24_{s}", [128, NK * 2 * D], bf16, kind="ExternalInput").ap() for s in (1, 2)]
    wmt_ap = nc.dram_tensor("wmt", [128, 16 * NK * 128], bf16, kind="ExternalInput").ap()
    bmap_ap = nc.dram_tensor("bmap", [128, 16], f32, kind="ExternalInput").ap()
    ident_ap = nc.dram_tensor("ident", [128, 128], f32r, kind="ExternalInput").ap()
    out_ap = nc.dram_tensor("out", [D2, R], bf16, kind="ExternalOutput").ap()

    with tile.TileContext(nc) as tc, ExitStack() as ctx:
        sb_x = ctx.enter_context(tc.tile_pool(name="sb_x", bufs=2))
        sb_xo = ctx.enter_context(tc.tile_pool(name="sb_xo", bufs=1))
        sb_w13 = ctx.enter_context(tc.tile_pool(name="sb_w13", bufs=16))
        sb_w24 = ctx.enter_context(tc.tile_pool(name="sb_w24", bufs=8))
        sb_wm = ctx.enter_context(tc.tile_pool(name="sb_wm", bufs=8))
        sb_ws = ctx.enter_context(tc.tile_pool(name="sb_ws", bufs=2))
        sb_st = ctx.enter_context(tc.tile_pool(name="sb_st", bufs=1))
        sb_rh = ctx.enter_context(tc.tile_pool(name="sb_rh", bufs=2))
        ps_att = ctx.enter_context(tc.tile_pool(name="ps_att", bufs=3, space="PSUM"))
        ps_big = ctx.enter_context(tc.tile_pool(name="ps_big", bufs=3, space="PSUM"))
        ps_sm = ctx.enter_context(tc.tile_pool(name="ps_sm", bufs=1, space="PSUM"))
        ps_bc = ctx.enter_context(tc.tile_pool(name="ps_bc", bufs=1, space="PSUM"))
        dram = ctx.enter_context(tc.tile_pool(name="dram", bufs=2, space="DRAM"))

        # ---- constants ----
        ones_row = sb_st.tile([1, 128], f32r, tag="ones_row")
        nc.vector.memset(ones_row[:].bitcast(f32), 1.0)
        ones_qb16 = sb_st.tile([Q_LEN, 1], bf16, tag="ones_qb16")
        nc.vector.memset(ones_qb16[:], 1.0)

        # ---- stage-1 input + const DMAs (sync queue: small stuff only) ----
        def load_stage_consts(s):
            vb = sb_st.tile([128, NK, 2], bf16, tag=f"vb{s}")
            nc.sync.dma_start(vb[:], vecb[s - 1][:].rearrange("p (c k) -> p c k", c=NK))
            sf = sb_st.tile([128, NK], f32, tag=f"sf{s}")
            nc.sync.dma_start(sf[:], sclf[s - 1][:])
            mT = sb_st.tile([128, NK, Q_LEN], bf16, tag=f"mT{s}")
            nc.sync.dma_start(mT[:], m_t[s - 1][:].rearrange("p (c q) -> p c q", c=NK))
            mN = sb_st.tile([Q_LEN, D], bf16, tag=f"mN{s}")
            nc.sync.dma_start(mN[:], m_n[s - 1][:])
            mask_i = sb_st.tile([Q_LEN, 1], i32, tag=f"mask_i{s}")
            nc.sync.dma_start(mask_i[:], msk[s - 1][:])
            return vb, sf, mT, mN, mask_i

        consts = {1: load_stage_consts(1)}
        ident = sb_st.tile([128, 128], f32r, tag="ident")
        nc.sync.dma_start(ident[:], ident_ap[:])
        consts[2] = load_stage_consts(2)

        # xt first on gpsimd so it gets HBM bandwidth; chunked so the first
        # scores matmuls can start while later chunks are still in flight.
        # Weights follow, the late ones gated behind the (E-dependent)
        # gpsimd max ops.
        xt0 = sb_x.tile([128, NK, R], bf16, tag="xt")
        for c in range(NK):
            nc.gpsimd.dma_start(xt0[:, c], xt_ap[:, c * R:(c + 1) * R])

        w13_t = {1: [], 2: []}
        w24_t = {1: [], 2: []}

        def load_w13(s, js, eng=None):
            eng = eng or nc.gpsimd
            for j in js:
                w13j = sb_w13.tile([128, 2, NK, 128], bf16, tag="w13")
                eng.dma_start(
                    w13j[:], w13[s - 1][:, j * 2048:(j + 1) * 2048]
                    .rearrange("p (t c m) -> p t c m", t=2, c=NK))
                w13_t[s].append(w13j)

        def load_w24(s, eng=None):
            eng = eng or nc.gpsimd
            for c in range(NK):
                w24c = sb_w24.tile([128, 2, D], bf16, tag="w24")
                eng.dma_start(
                    w24c[:], w24[s - 1][:, c * 2 * D:(c + 1) * 2 * D]
                    .rearrange("p (t m) -> p t m", t=2))
                w24_t[s].append(w24c)

        load_w13(1, range(0, 4), eng=nc.scalar)
        load_w13(1, range(4, NK), eng=nc.sync)
        load_w24(1, eng=nc.sync)

        # ---- per-stage prep: mst, memory_dot, mbias (runs early) ----
        def prep_stage(s):
            vb, sf, mT, mN, mask_i = consts[s]
            sfx = f"_s{s}"
            mst = sb_st.tile([128, NK, Q_LEN + 1], bf16, tag="mst" + sfx)
            nc.vector.tensor_copy(mst[:, :, 0:Q_LEN], mT[:])
            nc.vector.tensor_copy(mst[:, :, Q_LEN:Q_LEN + 1], vb[:, :, 0:1])
            for c in range(NK):
                nc.vector.tensor_scalar(mst[:, c, 0:Q_LEN], mst[:, c, 0:Q_LEN],
                                        sf[:, c:c + 1], None, Alu.mult)
            ps_md = ps_sm.tile([128, 4], f32, tag="ps_sm")
            for c in range(NK):
                nc.tensor.matmul(ps_md[0:Q_LEN, 0:1], mT[:, c], vb[:, c, 1:2],
                                 start=(c == 0), stop=(c == NK - 1))
            maskf = sb_st.tile([Q_LEN, 1], f32, tag="maskf" + sfx)
            nc.vector.tensor_copy(maskf[:], mask_i[:])
            mbias = sb_st.tile([Q_LEN, 1], f32, tag="mbias" + sfx)
            nc.vector.tensor_scalar(mbias[:], maskf[:], NEGBIG, -NEGBIG, Alu.mult, Alu.add)
            nc.vector.tensor_tensor(mbias[:], mbias[:], ps_md[0:Q_LEN, 0:1], Alu.add)
            return mst, mbias

        prep = {1: prep_stage(1)}
        wm_tiles = []
        bcol_all = sb_st.tile([128, 16], f32, tag="bcol_all")

        def run_stage(s, Xt):
            """One BiAttention stage; returns o^T tile [128, NK, R] bf16."""
            sfx = f"_s{s}"
            vb, sf, mT, mN, mask_i = consts[s]
            mst, mbias = prep[s]

            # ---------- scores for both row-halves ----------
            Es, eids = [], []
            for rh in range(RH):
                sl = slice(rh * 512, (rh + 1) * 512)
                ps_sc = ps_att.tile([Q_LEN + 1, 512], f32, tag="ps_att")
                for c in range(NK):
                    nc.tensor.matmul(ps_sc[:], mst[:, c], Xt[:, c, sl],
                                     start=(c == 0), stop=(c == NK - 1))
                E = sb_rh.tile([Q_LEN, 512], bf16, tag="E")
                nc.scalar.activation(E[:], ps_sc[0:Q_LEN], AF.Exp,
                                     bias=mbias[:], scale=1.0)
                eid = sb_rh.tile([1, 512], f32, tag="eid")
                nc.scalar.activation(eid[:], ps_sc[Q_LEN:Q_LEN + 1], AF.Exp)
                Es.append(E)
                eids.append(eid)

            # gpsimd max over q (for weight_two)
            mxs = []
            for rh in range(RH):
                mx = sb_rh.tile([Q_LEN, 512], f32, tag="mx")
                nc.gpsimd.partition_all_reduce(mx[:], Es[rh][:], Q_LEN, RedOp.max)
                mxs.append(mx)

            # column softmax sums in [128,4] layout, reciprocal, broadcast back
            P = sb_st.tile([Q_LEN, R], bf16, tag="P" + sfx)
            for rh in range(RH):
                E = Es[rh]
                ps_l1c = ps_sm.tile([128, 4], f32, tag="ps_sm")
                for q4 in range(4):
                    nc.tensor.matmul(ps_l1c[:, q4:q4 + 1],
                                     E[:, q4 * 128:(q4 + 1) * 128], ones_qb16[:],
                                     start=True, stop=True)
                l1r = sb_rh.tile([128, 4], f32r, tag="l1r")
                with nc.allow_low_precision(reason="softmax scale in f32r"):
                    nc.vector.reciprocal(l1r[:], ps_l1c[:])
                ps_rb = ps_bc.tile([128, 512], f32, tag="ps_bc")
                for q4 in range(4):
                    nc.tensor.matmul(
                        ps_rb[0:Q_LEN, q4 * 128:(q4 + 1) * 128],
                        l1r[:, q4:q4 + 1].broadcast_to([128, Q_LEN]),
                        ident[:], start=True, stop=True)
                nc.vector.tensor_tensor(P[:, rh * 512:(rh + 1) * 512],
                                        E[:], ps_rb[0:Q_LEN], Alu.mult)

            # weight_two per-column weights e2 (early, so the collective can
            # trigger as soon as possible; broadcast + partial sums on gpsimd)
            vh = sb_st.tile([128, 2 * NK], f32, tag="vh" + sfx)
            l2col = sb_st.tile([1, 2], f32, tag="l2col" + sfx)
            e2bs = []
            for rh in range(RH):
                e2 = sb_rh.tile([1, 512], bf16, tag="e2")
                nc.vector.tensor_tensor(e2[:], mxs[rh][0:1], eids[rh][:], Alu.mult)
                nc.vector.reduce_sum(l2col[:, rh:rh + 1], e2[:], AX.X)
                e2b = sb_rh.tile([128, 512], bf16, tag="e2b")
                nc.gpsimd.partition_broadcast(e2b[:], e2[:], 128)
                e2bs.append(e2b)

            # ---------- o1 / XO interleaved with W1-parts of early groups ---
            XO = sb_xo.tile([128, NK, R], bf16, tag="xo")
            oT = sb_x.tile([128, NK, R], bf16, tag="xt")
            w13s = w13_t[s]
            group_ps = {}

            def o1_pair(rh, c0):
                sl = slice(rh * 512, (rh + 1) * 512)
                for c in (c0, c0 + 1):
                    ps_o1 = ps_att.tile([128, 512], f32, tag="ps_att")
                    nc.tensor.matmul(ps_o1[:], mN[:, c * 128:(c + 1) * 128],
                                     P[:, sl], start=True, stop=True)
                    if c % 2 == 0:
                        nc.vector.tensor_tensor(XO[:, c, sl], Xt[:, c, sl],
                                                ps_o1[:], Alu.mult)
                    else:
                        # scalar-engine copy frees DVE (2x mode on bf16 pair)
                        o1s = sb_rh.tile([128, 512], bf16, tag="o1s")
                        nc.scalar.activation(o1s[:], ps_o1[:], AF.Copy)
                        nc.vector.tensor_tensor(XO[:, c, sl], Xt[:, c, sl],
                                                o1s[:], Alu.mult)

            def xpart(j, rh, cs):
                sl = slice(rh * 512, (rh + 1) * 512)
                if (j, rh) not in group_ps:
                    group_ps[(j, rh)] = ps_big.tile([128, 512], f32,
                                                    tag="ps_big", name="ps_ab")
                ps_ab = group_ps[(j, rh)]
                for c in cs:
                    nc.tensor.matmul(ps_ab[:], w13s[j][:, 0, c], Xt[:, c, sl],
                                     start=(c == 0), stop=False)

            def xoclose(j, rh, fuse_r64=False, Rsb=None):
                sl = slice(rh * 512, (rh + 1) * 512)
                ps_ab = group_ps.pop((j, rh))
                for c in range(NK):
                    nc.tensor.matmul(ps_ab[:], w13s[j][:, 1, c], XO[:, c, sl],
                                     start=False,
                                     stop=(c == NK - 1 and not fuse_r64))
                if fuse_r64:
                    nc.tensor.matmul(ps_ab[:], Rsb[:, j * 128:(j + 1) * 128],
                                     P[:, sl], start=False, stop=True)
                nc.scalar.activation(oT[:, j, sl], ps_ab[:], AF.Copy)

            o1_pair(0, 0)
            xpart(0, 0, range(0, 4))
            o1_pair(0, 2)
            xpart(0, 0, range(4, 8))
            o1_pair(0, 4)
            xpart(1, 0, range(0, 4))
            o1_pair(0, 6)
            xpart(1, 0, range(4, 8))
            o1_pair(1, 0)
            xpart(0, 1, range(0, 4))
            o1_pair(1, 2)
            xpart(0, 1, range(4, 8))
            o1_pair(1, 4)
            xoclose(0, 0)
            o1_pair(1, 6)
            xoclose(1, 0)
            xoclose(0, 1)

            def big_group(j, rh, fuse_r64=False, Rsb=None):
                xpart(j, rh, range(NK))
                xoclose(j, rh, fuse_r64=fuse_r64, Rsb=Rsb)

            big_group(1, 1)

            # ---------- weight-two stats (PE busy on big blocks) ----------
            # partial sums v = X^T e2: even chunks on gpsimd (idle anyway),
            # odd chunks on DVE (after XO in its queue).
            def stats_rh(rh):
                scrv = sb_rh.tile([128, 512], bf16, tag="scrv")
                sl = slice(rh * 512, (rh + 1) * 512)
                for c in range(NK):
                    nc.vector.scalar_tensor_tensor(
                        scrv[:], Xt[:, c, sl], 1.0, e2bs[rh][:],
                        Alu.mult, Alu.mult,
                        accum_out=vh[:, 2 * c + rh:2 * c + rh + 1])

            big_group(2, 0)
            stats_rh(0)
            big_group(2, 1)
            stats_rh(1)
            big_group(3, 0)

            # W2-part of R = M^T W2 (no collective dep) — early PE filler
            w24s = w24_t[s]
            ps_r = []
            for hf in range(2):
                ps_ri = ps_att.tile([128, 512], f32, tag="ps_att")
                ps_r.append(ps_ri)
                slh = slice(hf * 512, (hf + 1) * 512)
                for c in range(NK):
                    nc.tensor.matmul(ps_ri[0:Q_LEN], mT[:, c], w24s[c][:, 0, slh],
                                     start=(c == 0), stop=False)

            l2 = sb_st.tile([1, 1], f32, tag="l2" + sfx)
            nc.vector.reduce_sum(l2[:], l2col[:], AX.X)
            vsum = sb_st.tile([128, NK], f32, tag="vsum" + sfx)
            vh3 = vh[:].rearrange("p (c t) -> p c t", t=2)
            nc.vector.tensor_tensor(vsum[:], vh3[:, :, 0], vh3[:, :, 1], Alu.add)
            colsb = sb_st.tile([128, 16], f32, tag="colsb" + sfx)
            nc.vector.memset(colsb[:], 0.0)
            nc.vector.tensor_copy(colsb[:, 0:NK], vsum[:])
            nc.vector.tensor_copy(colsb[0:1, NK:NK + 1], l2[:])
            nc.vector.tensor_copy(colsb[0:1, NK + 1:NK + 2], l2[:])
            cin = dram.tile([128, 16], f32, tag="cin" + sfx)
            cout = dram.tile([128, 16], f32, tag="cout" + sfx)
            nc.sync.dma_start(cin[:], colsb[:])
            nc.gpsimd.collective_compute(
                "AllReduce", Alu.add,
                replica_groups=[[0, 1], [2, 3], [4, 5], [6, 7]],
                ins=[cin[:].opt()], outs=[cout[:].opt()])
            colg = sb_st.tile([128, 16], f32, tag="colg" + sfx)
            nc.sync.dma_start(colg[:], cout[:])

            big_group(3, 1)
            big_group(4, 0)
            big_group(4, 1)

            # prefetch next stage / final-linear weights + prep
            if s == 1:
                load_w13(2, range(NK))
                prep[2] = prep_stage(2)
            else:
                for j2 in range(16):
                    wmj = sb_wm.tile([128, NK, 128], bf16, tag="wm")
                    nc.sync.dma_start(
                        wmj[:], wmt_ap[:, j2 * 1024:(j2 + 1) * 1024]
                        .rearrange("p (c m) -> p c m", c=NK))
                    wm_tiles.append(wmj)
                nc.sync.dma_start(bcol_all[:], bmap_ap[:])

            # ---------- collective-dependent tail, PE kept dense ----------
            linv = sb_st.tile([1, 2], f32r, tag="linv" + sfx)
            with nc.allow_low_precision(reason="weight-two scale in f32r"):
                nc.vector.reciprocal(linv[:], colg[0:1, NK:NK + 2])
            ps_tb = ps_sm.tile([128, 4], f32, tag="ps_sm")
            nc.tensor.matmul(ps_tb[:, 0:2], ones_row[:], linv[:], start=True, stop=True)
            tvec = sb_st.tile([128, NK], f32, tag="tvec" + sfx)
            nc.vector.tensor_scalar(tvec[:], colg[:, 0:NK], ps_tb[:, 0:1], None, Alu.mult)
            w4sc = []
            for c in range(NK):
                w4c = sb_ws.tile([128, D], bf16, tag="w4sc")
                nc.vector.tensor_scalar(w4c[:], w24s[c][:, 1], tvec[:, c:c + 1],
                                        None, Alu.mult)
                w4sc.append(w4c)

            big_group(5, 0)
            big_group(5, 1)

            for hf in range(2):
                slh = slice(hf * 512, (hf + 1) * 512)
                for c in range(NK):
                    nc.tensor.matmul(ps_r[hf][0:Q_LEN], mT[:, c], w4sc[c][:, slh],
                                     start=False, stop=(c == NK - 1))
            Rsb = sb_st.tile([Q_LEN, D], bf16, tag="Rsb" + sfx)
            for hf in range(2):
                nc.scalar.activation(Rsb[:, hf * 512:(hf + 1) * 512],
                                     ps_r[hf][0:Q_LEN], AF.Copy)

            # rank-64 correction: j6/j7 fused in-group; j0..j5 via psum + add
            def r64(j, rh):
                sl = slice(rh * 512, (rh + 1) * 512)
                ps_c = ps_att.tile([128, 512], f32, tag="ps_att")
                nc.tensor.matmul(ps_c[:], Rsb[:, j * 128:(j + 1) * 128], P[:, sl],
                                 start=True, stop=True)
                nc.vector.tensor_tensor(oT[:, j, sl], oT[:, j, sl], ps_c[:], Alu.add)

            big_group(6, 0, fuse_r64=True, Rsb=Rsb)
            r64(0, 0)
            r64(1, 0)
            big_group(6, 1, fuse_r64=True, Rsb=Rsb)
            r64(2, 0)
            r64(3, 0)
            big_group(7, 0, fuse_r64=True, Rsb=Rsb)
            r64(4, 0)
            r64(5, 0)
            big_group(7, 1, fuse_r64=True, Rsb=Rsb)
            for j in range(6):
                r64(j, 1)
            return oT

        o1T = run_stage(1, xt0)
        load_w24(2)
        o2T = run_stage(2, o1T)

        # ---------- final linear (transposed): outT = w_mapT.T @ o2T + b ----
        for j2 in range(16):
            wmj = wm_tiles[j2]
            for rh in range(RH):
                sl = slice(rh * 512, (rh + 1) * 512)
                ps_f = ps_big.tile([128, 512], f32, tag="ps_big")
                for c in range(NK):
                    nc.tensor.matmul(ps_f[:], wmj[:, c], o2T[:, c, sl],
                                     start=(c == 0), stop=(c == NK - 1))
                outsb = sb_ws.tile([128, 512], bf16, tag="outsb")
                if (j2 + rh) % 2 == 0:
                    nc.scalar.activation(outsb[:], ps_f[:], AF.Identity,
                                         bias=bcol_all[:, j2:j2 + 1], scale=1.0)
                else:
                    nc.vector.tensor_scalar(outsb[:], ps_f[:],
                                            bcol_all[:, j2:j2 + 1], None, Alu.add)
                nc.sync.dma_start(out_ap[j2 * 128:(j2 + 1) * 128, sl], outsb[:])

    nc.compile()
    return nc


def _get_nc():
    global _CACHED_NC
    if _CACHED_NC is None:
        _CACHED_NC = _build_nc()
    return _CACHED_NC


def _bf(a):
    return np.ascontiguousarray(np.asarray(a, dtype=np.float32).astype(ml_dtypes.bfloat16))


def _shard_inputs(inputs):
    """Build the 8 per-core input maps (pure layout work, no arithmetic)."""
    x = np.asarray(inputs["ctx_features"], dtype=np.float32)
    q1 = np.asarray(inputs["sub_q1_features"], dtype=np.float32)
    q2 = np.asarray(inputs["sub_q2_features"], dtype=np.float32)
    k1 = np.ascontiguousarray(np.asarray(inputs["sub_q1_attn_mask"], dtype=np.int32))
    k2 = np.ascontiguousarray(np.asarray(inputs["sub_q2_attn_mask"], dtype=np.int32))

    def wpack13(w_out):
        # w_out [D, 4D] -> wb = w_out.T [4D, D]; W_k = wb[kD:(k+1)D]
        wb = np.asarray(w_out, dtype=np.float32).T
        W1, W3 = wb[0:D], wb[2 * D:3 * D]

        def v(W):  # [D_in, D_out] -> [c, p, j, m]
            return W.reshape(NK, 128, NK, 128)
        pk = np.stack([v(W1), v(W3)], axis=0)  # [t, c, p, j, m]
        pk = pk.transpose(2, 3, 0, 1, 4)       # [p, j, t, c, m]
        return _bf(pk.reshape(128, NK * 2 * NK * 128))

    def wpack24(w_out):
        wb = np.asarray(w_out, dtype=np.float32).T
        W2, W4 = wb[D:2 * D], wb[3 * D:4 * D]
        pk = np.stack([W2.reshape(NK, 128, D), W4.reshape(NK, 128, D)], axis=0)
        pk = pk.transpose(2, 1, 0, 3)          # [p, c, t, m]
        return _bf(pk.reshape(128, NK * 2 * D))

    wmT = np.asarray(inputs["w_map"], dtype=np.float32).T  # [D, 2D]
    wmt = wmT.reshape(NK, 128, 16, 128).transpose(1, 2, 0, 3)
    wmt = _bf(wmt.reshape(128, 16 * NK * 128))
    bmap = np.ascontiguousarray(
        np.asarray(inputs["b_map"], dtype=np.float32).reshape(16, 128).T)

    def ptile(vec_list, dtype):  # [D] vectors -> [128, NK*k] p-major
        v = np.stack([np.asarray(c, dtype=np.float32) for c in vec_list], axis=-1)
        k = v.shape[-1]
        out = v.reshape(NK, 128, k).transpose(1, 0, 2).reshape(128, NK * k)
        if dtype == "bf16":
            return _bf(out)
        return np.ascontiguousarray(out)

    stage_common = {
        "vecb1": ptile([inputs["w_in1"], inputs["w_mem1"]], "bf16"),
        "vecb2": ptile([inputs["w_in2"], inputs["w_mem2"]], "bf16"),
        "sclf1": ptile([inputs["scale1"]], "f32"),
        "sclf2": ptile([inputs["scale2"]], "f32"),
        "w13_1": wpack13(inputs["w_out1"]), "w13_2": wpack13(inputs["w_out2"]),
        "w24_1": wpack24(inputs["w_out1"]), "w24_2": wpack24(inputs["w_out2"]),
        "wmt": wmt, "bmap": bmap,
        "ident": np.ascontiguousarray(np.eye(128, dtype=np.float32)),
    }

    in_maps = []
    for core in range(N_CORES):
        b, h = divmod(core, 2)
        xT = x[b, h * R:(h + 1) * R, :].T  # [D, R]
        xt_tile = _bf(xT.reshape(NK, 128, R).transpose(1, 0, 2).reshape(128, NK * R))
        m = {}
        for s, q, kk in ((1, q1, k1), (2, q2, k2)):
            mT = q[b].T  # [D, Q]
            m[f"m{s}t"] = _bf(
                mT.reshape(NK, 128, Q_LEN).transpose(1, 0, 2).reshape(128, NK * Q_LEN))
            m[f"m{s}n"] = _bf(q[b])
            m[f"mask{s}"] = np.ascontiguousarray(kk[b].reshape(Q_LEN, 1))
        in_maps.append({"xt": xt_tile, **m, **stage_common})
    return in_maps


def _gather_outputs(results):
    out = np.empty((B, C_LEN, D2), dtype=np.float32)
    for core in range(N_CORES):
        b, h = divmod(core, 2)
        out[b, h * R:(h + 1) * R, :] = results[core]["out"].T.astype(np.float32)
    return out


def kernel(**inputs):
    nc = _get_nc()
    in_maps = _shard_inputs(inputs)
    last_err = None
    for _attempt in range(3):
        try:
            res = run_bass_kernel_spmd(nc, in_maps, core_ids=list(range(N_CORES)))
            return _gather_outputs(res.results)
        except Exception as e:  # transient device errors: retry
            last_err = e
    raise last_err
